# revision 7
# baseline (speedup 1.0000x reference)
"""Trainium2 Bass kernel for nn_DCTLinearFactored (v2: fp8 primary stream).

Math: reference computes
    coeff[b,i,j] = basis[i] @ x2d[b] @ basis[j]        (2D DCT)
    result[b]    = sum_ij coeff[b,i,j] w_h[i] w_v[j]
    out[b]       = sigmoid(result[b] + bias)

The rank-1 weight collapses the whole thing to a bilinear form
    result[b] = u^T x2d[b] v,   u = basis^T w_h,  v = basis^T w_v
i.e. one streaming pass over x -> HBM-bandwidth bound.  x is re-encoded
host-side (w-independent):
    x8   = e4m3(x)                  1 B/elt, dense primary stream
    res16= fp16(x - x8)             2 B/elt, fetched only for flagged rows

L1 (dense): DoubleRow fp8 matmuls of x8 against a 4-level fp8 stationary
representation of u (u ~ uq0 + uq1/S + uq2/S^2 + uq3/S^3) so the only L1
error is x-quantization (sigma ~12, max ~34 across rows).  Two x-tiles
share one PSUM accumulation group (M=32 stationary: 2 tile-parities x 4
levels x 4 batch slots), so VectorE folds (x v, reduce over l) run once
per 2 tiles on 32 partitions.

Flags: per 8-row group, fold levels via a tiny PE matmul, band-test
|s1 - mid| < hw on VectorE (rows whose sigmoid output is sensitive at L1
precision), build a bitmask with a pow2 matmul, values_load it on PE.

L2 (conditional, ~10% of rows): per flagged row, a predicated DMA
(scalar queue) fetches the row's fp16 residual in a row-major layout
[128 part, 2048], and a tc.If-gated block runs 4 fp16 matmuls with a
slot-masked [uhi|ulo] stationary (row's result lands at psum partitions
2c..2c+2, zeros elsewhere) + a VectorE fold accumulated into s2cols.
Unflagged rows contribute 0 (s2cols zero-initialized).  Final: fold
matmuls combine r1 levels (by tile parity) and s2cols into psF [4, NT],
ScalarE applies sigmoid(+bias), one small DMA out.
"""

import os

import numpy as np

N = 512
BATCH = 256
NCORES = 8
BPC = BATCH // NCORES          # batch rows per core = 32
TB = 4                         # batch rows per x-tile
NT = BPC // TB                 # x-tiles per core = 8
FREE = TB * N * N // 128       # free dim of an x-tile = 8192
NJ = FREE // 512               # 512-col slices per x-tile = 16
NG = NT // 2                   # psum groups (2 tiles each) = 4
NLVL = 4                       # fp8 levels representing u
SLVL = 64.0                    # inter-level scale
RFREE = N * N // 128           # free dim of a residual row = 2048
BAND_LO = -63.0                # sigmoid-sensitive band on L1 estimate
BAND_HI = 53.0

_CACHE = {}


def _dct_basis_np(n):
    u = np.arange(n)
    cu = np.where(u == 0, np.sqrt(1.0 / n), np.sqrt(2.0 / n))
    cos = np.cos((2.0 * u[:, None] + 1.0) * u[None, :] * np.pi / (2.0 * n))
    return (cu * cos).T.astype(np.float32)  # (n, n), row k = freq-k basis


def _build_nc():
    import concourse.bacc as bacc
    import concourse.bass as bass
    import concourse.mybir as mybir
    import concourse.tile as tile

    f32 = mybir.dt.float32
    i32 = mybir.dt.int32
    f16 = mybir.dt.float16
    f8 = mybir.dt.float8e4
    nc = bacc.Bacc(
        "TRN2", target_bir_lowering=False, debug=False, num_devices=NCORES
    )
    x8_h = nc.dram_tensor("x8", [NT, 128, FREE], f8, kind="ExternalInput")
    res_h = nc.dram_tensor("res", [BPC, 128, RFREE], f16, kind="ExternalInput")
    # uq: 16 DoubleRow stationary variants (par,jj), each [128, 2, 32] -> 64B
    uq_h = nc.dram_tensor("uq", [128, 16 * 64], f8, kind="ExternalInput")
    # non-DR stationary variants (par, j): 32 variants x M=32 cols
    uqnd_h = nc.dram_tensor("uqnd", [128, 32 * 32], f8, kind="ExternalInput")
    # u16m: 16 masked variants (c, j), each [128, 8]
    u16_h = nc.dram_tensor("u16", [128, 16 * 8], f16, kind="ExternalInput")
    v32_h = nc.dram_tensor("v32", [32, N], f32, kind="ExternalInput")
    # cst cols: fold32b [32,8] 0:8 | fold32_p0 [32,4] 8:12 | fold32_p1 12:16
    #   | fold8c [8,4] 16:20 | pow2 [8,1] 20 | bias [4,1] 21 | zeros [8,8] 22:30
    CW = 30
    cst_h = nc.dram_tensor("cst", [32, CW], f32, kind="ExternalInput")
    out_h = nc.dram_tensor("out", [TB, NT], f32, kind="ExternalOutput")

    MID = float(os.environ.get("K_MID", str((BAND_LO + BAND_HI) / 2)))
    HW = float(os.environ.get("K_HW", str((BAND_HI - BAND_LO) / 2)))
    XBUFS = int(os.environ.get("K_XBUFS", "5"))
    DR = int(os.environ.get("K_DR", "1"))
    RBUFS = int(os.environ.get("K_RBUFS", "16"))

    PEV = [mybir.EngineType.PE, mybir.EngineType.DVE]

    with tile.TileContext(nc) as tc:
        with (
            tc.tile_pool(name="const", bufs=1) as cpool,
            tc.tile_pool(name="xp", bufs=XBUFS) as xpool,
            tc.tile_pool(name="rp", bufs=RBUFS) as rpool,
            tc.tile_pool(name="sc", bufs=2) as spool,
            tc.tile_pool(name="psa", bufs=2, space=bass.MemorySpace.PSUM) as pspA,
            tc.tile_pool(name="psb", bufs=3, space=bass.MemorySpace.PSUM) as pspB,
            tc.tile_pool(name="pss", bufs=2, space=bass.MemorySpace.PSUM) as pspS,
            tc.tile_pool(name="psf", bufs=1, space=bass.MemorySpace.PSUM) as pspF,
        ):
            cst_t = cpool.tile([32, CW], f32)
            nc.sync.dma_start(cst_t[:], cst_h[:])
            uq_t = cpool.tile([128, 16 * 64], f8)
            nc.sync.dma_start(uq_t[:], uq_h[:])
            uqnd_t = cpool.tile([128, 32 * 32], f8)
            nc.sync.dma_start(uqnd_t[:], uqnd_h[:])
            u16_t = cpool.tile([128, 16 * 8], f16)
            nc.sync.dma_start(u16_t[:], u16_h[:])
            v32_t = cpool.tile([32, N], f32)
            nc.sync.dma_start(v32_t[:], v32_h[:])

            fold32b = cst_t[0:32, 0:8]
            fold32p = [cst_t[0:32, 8:12], cst_t[0:32, 12:16]]
            fold8c = cst_t[0:8, 16:20]
            pow2_t = cst_t[0:8, 20:21]
            b4_t = cst_t[0:4, 21:22]
            zro_t = cst_t[0:8, 22:30]
            v8_t = v32_t[0:8, 0:N]

            r1cols = cpool.tile([32, NG], f32)
            s2cols = cpool.tile([8, NT], f32)
            flagf = cpool.tile([8, NG], f32)
            flagi = cpool.tile([8, NG], i32)
            bm_i = cpool.tile([1, NG], i32)
            o_all = cpool.tile([TB, NT], f32)

            # zero-init s2cols (unflagged rows contribute 0)
            nc.vector.tensor_copy(out=s2cols[:], in_=zro_t)

            res_tiles = {}

            def emit_row_ifs(g):
                bm = nc.values_load(
                    bm_i[0:1, g:g + 1],
                    engines=PEV,
                    min_val=0,
                    max_val=255,
                    skip_runtime_bounds_check=True,
                )
                for i in range(8):
                    resr = res_tiles.pop((g, i))
                    t = 2 * g + i // 4
                    c = i % 4
                    psB = pspB.tile([8, 512], f32, tag="psB")
                    scB = spool.tile([8, 512], f32, tag="scB")
                    r2row = spool.tile([8, 1], f32, tag="r2row")
                    with tc.If((bm & (1 << i)) != 0):
                        for j in range(4):
                            vsel = 8 * (4 * c + j)
                            nc.tensor.matmul(
                                psB[:],
                                u16_t[:, vsel:vsel + 8],
                                resr[:, 512 * j:512 * (j + 1)],
                                start=(j == 0),
                                stop=(j == 3),
                            )
                        nc.vector.tensor_tensor(
                            out=scB[:], in0=psB[:], in1=v8_t,
                            op=mybir.AluOpType.mult,
                        )
                        nc.vector.tensor_reduce(
                            out=r2row[:],
                            in_=scB[:],
                            axis=mybir.AxisListType.X,
                            op=mybir.AluOpType.add,
                        )
                        nc.vector.tensor_tensor(
                            out=s2cols[:, t:t + 1],
                            in0=s2cols[:, t:t + 1],
                            in1=r2row[:],
                            op=mybir.AluOpType.add,
                        )

            for g in range(NG):
                psA = pspA.tile([32, 512], f32, tag="psA")
                for par in range(2):
                    t = 2 * g + par
                    xh = xpool.tile([128, FREE], f8)
                    nc.sync.dma_start(xh[:], x8_h[t, :, :])
                    if DR:
                        for jj in range(NJ // 2):
                            vsel = 64 * (8 * par + jj)
                            nc.tensor.matmul(
                                psA[:],
                                uq_t[:, vsel:vsel + 64].rearrange(
                                    "p (s m) -> p s m", s=2
                                ),
                                xh[:, 1024 * jj:1024 * (jj + 1)].rearrange(
                                    "p (s l) -> p s l", s=2
                                ),
                                start=(par == 0 and jj == 0),
                                stop=(par == 1 and jj == NJ // 2 - 1),
                                perf_mode=mybir.MatmulPerfMode.DoubleRow,
                            )
                    else:
                        for j in range(NJ):
                            vsel = 32 * (16 * par + j)
                            nc.tensor.matmul(
                                psA[:],
                                uqnd_t[:, vsel:vsel + 32],
                                xh[:, 512 * j:512 * (j + 1)],
                                start=(par == 0 and j == 0),
                                stop=(par == 1 and j == NJ - 1),
                            )
                    if par == 0 and g > 0:
                        # interleave previous group's flag matmul while
                        # this group's second tile streams
                        psS = pspS.tile([8, 1], f32, tag="psS")
                        nc.tensor.matmul(
                            psS[:], fold32b, r1cols[:, g - 1:g],
                            start=True, stop=True,
                        )
                        s1ab = spool.tile([8, 1], f32, tag="s1ab")
                        nc.vector.tensor_scalar(
                            out=s1ab[:], in0=psS[:],
                            scalar1=MID, scalar2=None,
                            op0=mybir.AluOpType.subtract,
                        )
                        nc.vector.tensor_tensor(
                            out=s1ab[:], in0=s1ab[:], in1=s1ab[:],
                            op=mybir.AluOpType.mult,
                        )
                        nc.vector.tensor_scalar(
                            out=flagf[:, g - 1:g], in0=s1ab[:],
                            scalar1=HW * HW, scalar2=None,
                            op0=mybir.AluOpType.is_lt,
                        )
                        nc.vector.tensor_copy(
                            out=flagi[:, g - 1:g], in_=flagf[:, g - 1:g]
                        )
                # group DVE fold
                scA = spool.tile([32, 512], f32, tag="scA")
                nc.vector.tensor_tensor(
                    out=scA[:], in0=psA[:], in1=v32_t,
                    op=mybir.AluOpType.mult,
                )
                nc.vector.tensor_reduce(
                    out=r1cols[:, g:g + 1],
                    in_=scA[:],
                    axis=mybir.AxisListType.X,
                    op=mybir.AluOpType.add,
                )
                if g > 0:
                    # bitmask matmul + copies + residual prefetch for g-1
                    psM = pspS.tile([1, 1], f32, tag="psS")
                    nc.tensor.matmul(
                        psM[:], pow2_t, flagf[:, g - 1:g],
                        start=True, stop=True,
                    )
                    nc.vector.tensor_copy(out=bm_i[0:1, g - 1:g], in_=psM[:])
                    for i in range(8):
                        reg = nc.values_load(
                            flagi[i:i + 1, g - 1:g],
                            engines=[mybir.EngineType.Activation],
                            min_val=0,
                            max_val=1,
                            skip_runtime_bounds_check=True,
                        )
                        resr = rpool.tile([128, RFREE], f16)
                        res_tiles[(g - 1, i)] = resr
                        nc.scalar.dma_start(
                            resr[:], res_h[8 * (g - 1) + i, :, :],
                            cond=reg, cond_hint=False,
                        )
                if g > 1:
                    emit_row_ifs(g - 2)

            # tail: flags for the last group
            psS = pspS.tile([8, 1], f32, tag="psS")
            nc.tensor.matmul(
                psS[:], fold32b, r1cols[:, NG - 1:NG], start=True, stop=True
            )
            s1ab = spool.tile([8, 1], f32, tag="s1ab")
            nc.vector.tensor_scalar(
                out=s1ab[:], in0=psS[:], scalar1=MID, scalar2=None,
                op0=mybir.AluOpType.subtract,
            )
            nc.vector.tensor_tensor(
                out=s1ab[:], in0=s1ab[:], in1=s1ab[:],
                op=mybir.AluOpType.mult,
            )
            nc.vector.tensor_scalar(
                out=flagf[:, NG - 1:NG], in0=s1ab[:],
                scalar1=HW * HW, scalar2=None,
                op0=mybir.AluOpType.is_lt,
            )
            nc.vector.tensor_copy(
                out=flagi[:, NG - 1:NG], in_=flagf[:, NG - 1:NG]
            )
            psM = pspS.tile([1, 1], f32, tag="psS")
            nc.tensor.matmul(
                psM[:], pow2_t, flagf[:, NG - 1:NG], start=True, stop=True
            )
            nc.vector.tensor_copy(out=bm_i[0:1, NG - 1:NG], in_=psM[:])
            for i in range(8):
                reg = nc.values_load(
                    flagi[i:i + 1, NG - 1:NG],
                    engines=[mybir.EngineType.Activation],
                    min_val=0, max_val=1, skip_runtime_bounds_check=True,
                )
                resr = rpool.tile([128, RFREE], f16)
                res_tiles[(NG - 1, i)] = resr
                nc.scalar.dma_start(
                    resr[:], res_h[8 * (NG - 1) + i, :, :],
                    cond=reg, cond_hint=False,
                )
            emit_row_ifs(NG - 2)
            emit_row_ifs(NG - 1)

            # final fold + sigmoid + out
            psF = pspF.tile([TB, NT], f32, tag="psF")
            psFv = psF.rearrange("p (g q) -> p q g", q=2)
            nc.tensor.matmul(psF[:], fold8c, s2cols[:], start=True, stop=False)
            nc.tensor.matmul(
                psFv[:, 0, :], fold32p[0], r1cols[:],
                start=False, stop=False, skip_group_check=True,
            )
            nc.tensor.matmul(
                psFv[:, 1, :], fold32p[1], r1cols[:],
                start=False, stop=True, skip_group_check=True,
            )
            nc.scalar.activation(
                o_all[:],
                psF[:],
                mybir.ActivationFunctionType.Sigmoid,
                bias=b4_t,
            )
            nc.sync.dma_start(out_h[:], o_all[:])
    nc.compile()
    return nc


def _get_nc():
    if "nc" not in _CACHE:
        _CACHE["nc"] = _build_nc()
    return _CACHE["nc"]


def _host_prep(x, w_horizontal, w_vertical, bias):
    import ml_dtypes

    f8 = ml_dtypes.float8_e4m3
    basis = _dct_basis_np(N).astype(np.float64)  # (n, n) row k = freq k
    u = (np.asarray(w_horizontal, np.float64) @ basis).astype(np.float64)
    v = (np.asarray(w_vertical, np.float64) @ basis).astype(np.float32)
    bias_v = float(np.asarray(bias).reshape(-1)[0])

    # 4-level fp8 representation of u (scaled residual chain)
    uqs = []
    du = u.copy()
    for lvl in range(NLVL):
        q = (du * SLVL ** lvl).astype(f8)
        uqs.append(q.astype(np.float64))
        du = du - q.astype(np.float64) / SLVL ** lvl
    uhi = u.astype(np.float16)
    ulo = (u - uhi.astype(np.float64)).astype(np.float16)

    # DR stationary variants: st[p, 64*(8*par+jj) + 32*s + 16*par + 4*lvl + c]
    #   = uqs[lvl][16*(p%32) + 2*jj + s]   if p//32 == c else 0
    uq = np.zeros((128, 16 * 64), np.float32)
    qq = np.arange(32)
    for c in range(TB):
        p = 32 * c + qq
        for par in range(2):
            for jj in range(NJ // 2):
                base = 64 * (8 * par + jj)
                for s in range(2):
                    for lvl in range(NLVL):
                        col = base + 32 * s + 16 * par + 4 * lvl + c
                        uq[p, col] = uqs[lvl][16 * qq + 2 * jj + s]
    uq = uq.astype(f8)

    # non-DR stationary: uqnd[32c+q, 32*(16*par+j) + 16*par+4*lvl+c]
    uqnd = np.zeros((128, 32 * 32), np.float32)
    for c in range(TB):
        p = 32 * c + qq
        for par in range(2):
            for j in range(NJ):
                base = 32 * (16 * par + j)
                for lvl in range(NLVL):
                    uqnd[p, base + 16 * par + 4 * lvl + c] = uqs[lvl][
                        16 * qq + j
                    ]
    uqnd = uqnd.astype(f8)

    # L2 stationary variants (c, j): st[p, 8*(4c+j) + 2c'+h]
    #   = (uhi|ulo)[4p+j] if c'==c else 0
    u16 = np.zeros((128, 16 * 8), np.float32)
    pp = np.arange(128)
    for c in range(TB):
        for j in range(4):
            base = 8 * (4 * c + j)
            u16[pp, base + 2 * c] = uhi.astype(np.float32)[4 * pp + j]
            u16[pp, base + 2 * c + 1] = ulo.astype(np.float32)[4 * pp + j]
    u16 = u16.astype(np.float16)

    v32 = np.broadcast_to(v[None, :], (32, N)).copy()

    CW = 30
    cst = np.zeros((32, CW), np.float32)
    # fold32b [32,8]: st[16*par+4*lvl+c, 4*par+c] = SLVL**-lvl
    # fold32p[par] [32,4]: st[16*par+4*lvl+c, c] = SLVL**-lvl
    for par in range(2):
        for lvl in range(NLVL):
            for c in range(TB):
                cst[16 * par + 4 * lvl + c, 4 * par + c] = SLVL ** -lvl
                cst[16 * par + 4 * lvl + c, 8 + 4 * par + c] = SLVL ** -lvl
    # fold8c [8,4]: st[2*c+h, 16+c] = 1
    for c in range(TB):
        cst[2 * c, 16 + c] = 1.0
        cst[2 * c + 1, 16 + c] = 1.0
    cst[0:8, 20] = [1.0, 2.0, 4.0, 8.0, 16.0, 32.0, 64.0, 128.0]
    cst[0:4, 21] = bias_v
    # zeros block cols 22:30 stays 0

    x = np.ascontiguousarray(np.asarray(x, np.float32))
    x8 = x.astype(f8)
    res = x - x8.astype(np.float32)
    in_maps = []
    for i in range(NCORES):
        sl = slice(i * BPC, (i + 1) * BPC)
        x8c = x8[sl].reshape(NT, 128, FREE)
        # residual row layout: [row, p, 512*j+l] = res2d[4p+j, l]
        resc = res[sl].astype(np.float16).reshape(BPC, 128, RFREE)
        in_maps.append(
            {
                "x8": x8c,
                "res": resc,
                "uq": uq,
                "uqnd": uqnd,
                "u16": u16,
                "v32": v32,
                "cst": cst,
            }
        )
    return in_maps


def _run(x, w_horizontal, w_vertical, bias, trace=False):
    from concourse.bass_utils import run_bass_kernel_spmd

    nc = _get_nc()
    in_maps = _host_prep(x, w_horizontal, w_vertical, bias)
    res = run_bass_kernel_spmd(
        nc, in_maps, core_ids=list(range(NCORES)), trace=trace
    )
    # out[c, t] holds batch row b = 4*t + c of this core's shard
    parts = [
        np.asarray(res.results[i]["out"]).T.reshape(BPC) for i in range(NCORES)
    ]
    full = np.concatenate(parts).astype(np.float32)[:, None]
    return full, res


def kernel(x, w_horizontal, w_vertical, bias):
    out, _ = _run(x, w_horizontal, w_vertical, bias, trace=False)
    return out
